# revision 17
# baseline (speedup 1.0000x reference)
"""Trainium2 Bass kernel for DEMA (double exponential moving average) decomposition.

reference semantics (per batch row b, channel c, over time t):
    s0 = x[0], b0 = x[1] - x[0]
    for t in 1..T-1:
        s_t = alpha*x_t + (1-alpha)*(s_{t-1} + b_{t-1})
        b_t = beta*(s_t - s_{t-1}) + (1-beta)*b_{t-1}
    ma = [s0, s1, ..., s_{T-1}];  res = x - ma;  returns (res, ma)

The recurrence is linear in x.  Time is split into 7 blocks (6x110 + 108,
block length capped by the 128-partition contraction limit).  With
z_t = (s_t, b_t), one stationary matrix per block computes, in a single
fp16 TensorE matmul over contraction rows [x_block (110); Z_k (2)]:

    out rows 0:2   = Z_{k+1}   (carry state entering the next block)
    out rows 2:112 = ma rows of this block

The schedule is k-major: all 8 batch-pairs of block k run back-to-back,
so a block's carry (rows 0:2, cast to fp16 by the PSUM->SBUF copy that
the ma rows need anyway) is placed into the next block's x tile by a
small SWDGE DMA while the other pairs' matmuls fill the PE.  This
removes the baseline's 7 V-matmuls per pair (a second full pass of x
through the PE) and drops PE work from 15x512 to 7x512 moving columns
per pair.

res = x - ma is reassembled on the host from the exact f32 input, so
only ma is stored from the device (halves store traffic).

Sharding: batch dim (128) split across 8 cores (16 rows each); the
recurrence runs only over time so no cross-core communication is needed.
"""

import numpy as np
from contextlib import ExitStack

import bass_rust as _bass_rust
import concourse.bass as bass
import concourse.tile as tile
from concourse import mybir
from concourse.bass_utils import run_bass_kernel_spmd


N_CORES = 8
B, T, C = 128, 768, 256
BS = B // N_CORES      # 16 batch rows per core
NPAIR = BS // 2        # 8 pairs, processed as 2 halves of 4
NH = 2                 # halves
PH = NPAIR // NH       # pairs per half
LB = [110, 110, 110, 110, 110, 110, 108]
NBK = len(LB)
LMAX = 110
KP = LMAX + 2          # contraction rows: 110 x + 2 carry
OFF = np.cumsum([0] + LB)
F32 = mybir.dt.float32
F16 = mybir.dt.float16

# packed stationary tensor [112, 334]: [S0 | Sgen | Slast] (transposed mats)
_WCOLS = 3 * KP - 2


def _dema_mats(alpha: float, beta: float):
    """Per-block stationary matrices M_k [2+L, 112] in float64.

    M_k rows 0:2 produce Z_{k+1}; rows 2:2+L produce the ma rows.
    Contraction cols 0:2 are the carry Z_k, cols 2:112 the x rows (so the
    carry lands at partition base 0 everywhere and the carry placement can
    be a plain engine copy instead of a DMA).
    """
    a, b = float(alpha), float(beta)
    A = np.array([[1.0 - a, 1.0 - a], [-a * b, b * (1.0 - a) + (1.0 - b)]])
    c = np.array([a, a * b])
    e_s = np.array([1.0, 0.0])
    Apow = [np.eye(2)]
    for _ in range(T + 2):
        Apow.append(A @ Apow[-1])
    M0 = np.array([[1.0, 0.0], [-1.0, 1.0]])

    mats = []
    for k, L in enumerate(LB):
        M = np.zeros((2 + L, KP))
        if k == 0:
            zc = Apow[L - 1] @ M0
            M[0:2, 2 + 0] = zc[:, 0]
            M[0:2, 2 + 1] = zc[:, 1]
            for i in range(1, L):
                M[0:2, 2 + i] += Apow[L - 1 - i] @ c
            M[2 + 0, 2 + 0] = 1.0
            for t in range(1, L):
                zc = Apow[t] @ M0
                M[2 + t, 2 + 0] += (e_s @ zc)[0]
                M[2 + t, 2 + 1] += (e_s @ zc)[1]
                for i in range(1, t + 1):
                    M[2 + t, 2 + i] += (Apow[t - i] @ c)[0]
        else:
            if k < NBK - 1:
                for ip in range(L):
                    M[0:2, 2 + ip] = Apow[L - 1 - ip] @ c
                M[0:2, 0:2] = Apow[L]
            for tp in range(L):
                for ip in range(tp + 1):
                    M[2 + tp, 2 + ip] = (Apow[tp - ip] @ c)[0]
                M[2 + tp, 0:2] = e_s @ Apow[tp + 1]
        mats.append(M)
    return mats


def _pack_weights(alpha: float, beta: float) -> np.ndarray:
    mats = _dema_mats(alpha, beta)
    wts = np.zeros((KP, _WCOLS), dtype=np.float16)
    wts[:, 0:KP] = mats[0].T                            # S0   [112, 112]
    wts[:, KP : 2 * KP] = mats[1].T                     # Sgen [112, 112]
    wts[:, 2 * KP : 2 * KP + LMAX] = mats[6].T          # Slast[112, 110]
    return wts


def _wait_limit(inst) -> int:
    # walrus in this container rejects >1 sync wait on several instruction
    # formats (S3_LW, DMA DIRECT2D, CTRL); keep a single wait everywhere
    return 1


class _SplitDrainTC(tile.TileContext):
    """This walrus build rejects more than a couple of sync waits per
    instruction.  After scheduling + the stock kernel-tail drain, walk every
    block and move excess waits onto injected same-engine nops placed
    immediately before the over-limit instruction (waits execute on the
    engine sequencer before dispatch, so this is semantics-preserving)."""

    def _drain_and_barrier(self, tick_clock, wait_clock):
        super()._drain_and_barrier(tick_clock, wait_clock)
        self._split_excess_waits()

    def _split_excess_waits(self):
        nc = self.nc
        cur_list = nc.cur_bb.bb.instructions if nc.cur_bb is not None else None
        for fn in nc.m.functions:
            for blk in fn.blocks:
                insts = blk.instructions
                i = 0
                while i < len(insts):
                    inst = insts[i]
                    si = getattr(inst, "sync_info", None)
                    waits = list(si.on_wait) if si is not None else []
                    limit = _wait_limit(inst)
                    if len(waits) <= limit:
                        i += 1
                        continue
                    keep = waits[:limit]
                    excess = waits[limit:]
                    nops = []
                    for j in range(0, len(excess)):
                        nop = nc.engines[inst.engine].nop(nofuse=True).ins
                        # engine.nop() appended to the current bb; relocate it
                        if cur_list is not None and cur_list and cur_list[-1] is nop:
                            cur_list.pop()
                        nop.sync_info = _bass_rust.SyncInfo(
                            on_wait=excess[j : j + 1], on_update=[]
                        )
                        nops.append(nop)
                    si.on_wait = keep
                    insts[i:i] = nops
                    i += len(nops) + 1


def _dram_ap(dram, k: int, h: int, nrows: int):
    """[nrows (partition=trow), PH*2*C] view of dram[k, 0:nrows, h]."""
    inner = PH * 2 * C
    return bass.AP(
        tensor=dram,
        offset=(k * LMAX * NH + h) * inner,
        ap=[[NH * inner, nrows], [1, inner]],
    )


def _build_nc() -> bass.Bass:
    nc = bass.Bass(trn_type="TRN2", target_bir_lowering=False, debug=False,
                   num_devices=N_CORES)
    # DRAM layout [k, trow, h, q, b', c]: per-(k,h) transfers are
    # [110 partitions x 4KB contiguous] HWDGE-friendly runs; the host does
    # the (cheap) permutation to/from [b, t, c] during shard/unshard.
    # Block 6 trows 108:110 are zero-padded by the host.
    x_d = nc.dram_tensor("x", (NBK, LMAX, NH, PH, 2, C), F16,
                         kind="ExternalInput")
    w_d = nc.dram_tensor("wts", (KP, _WCOLS), F16, kind="ExternalInput")
    z_d = nc.dram_tensor("z0", (2, NPAIR, 2, C), F16, kind="ExternalInput")
    ma_d = nc.dram_tensor("ma", (NBK, LMAX, NH, PH, 2, C), F16,
                          kind="ExternalOutput")

    with _SplitDrainTC(nc) as tc, ExitStack() as ctx:
        const = ctx.enter_context(tc.tile_pool(name="const", bufs=1))
        xbp = ctx.enter_context(tc.tile_pool(name="xb", bufs=NBK * NH))
        masp = ctx.enter_context(tc.tile_pool(name="mas", bufs=10))
        mps = ctx.enter_context(tc.tile_pool(name="mpsum", bufs=8,
                                             space="PSUM"))

        wts = const.tile([KP, _WCOLS], F16)
        s0 = wts[:, 0:KP]                   # block 0: [112, 112]
        sgen = wts[:, KP : 2 * KP]          # blocks 1..5: [112, 112]
        slast = wts[:, 2 * KP : 2 * KP + LMAX]  # block 6: [112, 110]

        # One x tile per block: partitions 0:2 carry Z_k, 2:112 the x rows.
        # All loads ride the SyncE HWDGE ring in order, leading the PE by
        # several blocks.  Block 0's carry rows are zeros (the S0 stationary
        # has zero coefficients there, but SBUF garbage could be NaN and
        # 0*NaN would poison the PE).  Block 0's x is loaded in 4 chunks:
        # region-level hazard tracking lets pair q's first matmul start as
        # soon as its own columns have landed.
        xbs = {}
        for k in range(NBK):
            xbs[k] = xbp.tile([KP, NPAIR, 2, C], F16, tag="xb", name=f"xb{k}")
        nc.scalar.dma_start(xbs[0][0:2, :], z_d.ap())
        nc.scalar.dma_start(wts[:], w_d.ap())
        inner = NPAIR * 2 * C
        for lo, hi in ((0, 1), (1, 2), (2, 5), (5, 8)):
            nc.sync.dma_start(
                xbs[0][2:KP, lo:hi],
                bass.AP(tensor=x_d, offset=lo * 2 * C,
                        ap=[[inner, LMAX], [1, (hi - lo) * 2 * C]]),
            )
        for k in range(1, NBK):
            nc.sync.dma_start(
                xbs[k][2:KP, :],
                bass.AP(tensor=x_d, offset=k * LMAX * inner,
                        ap=[[inner, LMAX], [1, inner]]),
            )

        # k-major: 8 matmuls per block (one per pair), PSUM bank = pair.
        # Each pair's PSUM output: rows 0:2 = Z_{k+1}, rows 2:112 = ma.
        # Two copies per pair chase the matmul: the full-tile fp16 copy into
        # mas (round-robin Vector/Scalar/GpSimd) and a tiny carry copy of
        # rows 0:2 straight into the next block's x tile (alternating
        # Vector/Scalar, kept off the engine doing this pair's main copy so
        # it dispatches the moment the matmul retires).  Block k+1's
        # matmuls therefore wait only on 4 tiny copies that complete well
        # inside the other half's matmul window.
        ncop = 0
        for k in range(NBK):
            last = k == NBK - 1
            nout = 2 + LB[k]
            xb = xbs[k]
            for h in range(NH):
                mas = masp.tile([KP, PH, 2, C], F16, tag="mas",
                                name=f"mas{k}_{h}")
                for q in range(PH):
                    mp = mps.tile([KP, 2, C], F32, tag="mp")
                    st = s0 if k == 0 else (slast if last else sgen)
                    nc.tensor.matmul(mp[0:nout], st, xb[:, h * PH + q],
                                     start=True, stop=True)
                    # the ONLY PSUM readers are these 28-per-engine main
                    # copies: more PSUM traffic (e.g. per-pair carry reads)
                    # contends with the PE's PSUM writes and slows the
                    # matmuls from ~600ns to ~760ns
                    if ncop % 2 == 0:
                        nc.vector.tensor_copy(mas[0:nout, q], mp[0:nout])
                    else:
                        nc.scalar.copy(mas[0:nout, q], mp[0:nout])
                    ncop += 1
                    if not last and q % 2 == 1:
                        # carry placement at 2-pair granularity, SBUF->SBUF
                        # from mas, on the engine that did NOT run the main
                        # copy that just retired.  The first half's carry
                        # lands while pairs q+1.. still matmul, so the next
                        # block's early matmuls (which only read their own
                        # columns, thanks to region-level hazard tracking)
                        # never wait on the late carry.
                        zdst = xbs[k + 1][0:2, h * PH + q - 1 : h * PH + q + 1]
                        if ncop % 2 == 0:
                            nc.scalar.copy(zdst, mas[0:2, q - 1 : q + 1])
                        else:
                            nc.vector.tensor_copy(zdst, mas[0:2, q - 1 : q + 1])
                # ma store rows 2:112 (DMA is exempt from the 32-aligned
                # partition-base rule); round-robin the three DGE rings (the
                # SyncE ring is drained of loads by the time stores flow)
                dst = _dram_ap(ma_d, k, h, LB[k])
                src = mas[2 : 2 + LB[k], :]
                ring = (k * NH + h) % 3
                if ring == 0:
                    nc.scalar.dma_start(dst, src)
                elif ring == 1:
                    nc.sync.dma_start(dst, src)
                else:
                    nc.gpsimd.dma_start(dst, src)

    return nc


_NC_CACHE: bass.Bass | None = None


def _get_nc() -> bass.Bass:
    global _NC_CACHE
    if _NC_CACHE is None:
        _NC_CACHE = _build_nc()
    return _NC_CACHE


def _tile_layout(x_shard: np.ndarray) -> np.ndarray:
    """[BS, T, C] fp16 -> [NBK, 110, NH, PH, 2, C] tile layout (zero-pad)."""
    v = x_shard.reshape(NH, PH, 2, T, C)
    out = np.zeros((NBK, LMAX, NH, PH, 2, C), dtype=np.float16)
    for k in range(NBK):
        chunk = v[:, :, :, OFF[k] : OFF[k] + LB[k], :]
        out[k, 0 : LB[k]] = chunk.transpose(3, 0, 1, 2, 4)
    return out


def _untile_layout(t: np.ndarray) -> np.ndarray:
    """[NBK, 110, NH, PH, 2, C] -> [BS, T, C] f32."""
    ma = np.empty((NH, PH, 2, T, C), dtype=np.float32)
    for k in range(NBK):
        ma[:, :, :, OFF[k] : OFF[k] + LB[k], :] = (
            t[k, 0 : LB[k]].astype(np.float32).transpose(1, 2, 3, 0, 4)
        )
    return ma.reshape(BS, T, C)


def kernel(x: np.ndarray, alpha, beta):
    x = np.asarray(x, dtype=np.float32)
    assert x.shape == (B, T, C), x.shape
    x16 = x.astype(np.float16)
    wts = _pack_weights(float(alpha), float(beta))

    nc = _get_nc()
    z0 = np.zeros((2, NPAIR, 2, C), dtype=np.float16)
    in_maps = [
        {"x": _tile_layout(x16[i * BS : (i + 1) * BS]), "wts": wts, "z0": z0}
        for i in range(N_CORES)
    ]
    out = run_bass_kernel_spmd(nc, in_maps, core_ids=list(range(N_CORES)))
    ma = np.concatenate(
        [_untile_layout(out.results[i]["ma"]) for i in range(N_CORES)], axis=0
    )
    res = x - ma
    return res, ma


# revision 18
# speedup vs baseline: 1.1987x; 1.1987x over previous
"""Trainium2 Bass kernel for DEMA (double exponential moving average) decomposition.

reference semantics (per batch row b, channel c, over time t):
    s0 = x[0], b0 = x[1] - x[0]
    for t in 1..T-1:
        s_t = alpha*x_t + (1-alpha)*(s_{t-1} + b_{t-1})
        b_t = beta*(s_t - s_{t-1}) + (1-beta)*b_{t-1}
    ma = [s0, s1, ..., s_{T-1}];  res = x - ma;  returns (res, ma)

The recurrence is linear in x.  Time is split into 7 blocks (6x110 + 108,
block length capped by the 128-partition contraction limit).  With
z_t = (s_t, b_t), one stationary matrix per block computes, in a single
fp16 TensorE matmul over contraction rows [x_block (110); Z_k (2)]:

    out rows 0:2   = Z_{k+1}   (carry state entering the next block)
    out rows 2:112 = ma rows of this block

The schedule is k-major: all 8 batch-pairs of block k run back-to-back,
so a block's carry (rows 0:2, cast to fp16 by the PSUM->SBUF copy that
the ma rows need anyway) is placed into the next block's x tile by a
small SWDGE DMA while the other pairs' matmuls fill the PE.  This
removes the baseline's 7 V-matmuls per pair (a second full pass of x
through the PE) and drops PE work from 15x512 to 7x512 moving columns
per pair.

res = x - ma is reassembled on the host from the exact f32 input, so
only ma is stored from the device (halves store traffic).

Sharding: batch dim (128) split across 8 cores (16 rows each); the
recurrence runs only over time so no cross-core communication is needed.
"""

import numpy as np
from contextlib import ExitStack

import bass_rust as _bass_rust
import concourse.bass as bass
import concourse.tile as tile
from concourse import mybir
from concourse.bass_utils import run_bass_kernel_spmd


N_CORES = 8
B, T, C = 128, 768, 256
BS = B // N_CORES      # 16 batch rows per core
NPAIR = BS // 2        # 8 pairs, processed as 2 halves of 4
NH = 2                 # halves
PH = NPAIR // NH       # pairs per half
LB = [110, 110, 110, 110, 110, 110, 108]
NBK = len(LB)
LMAX = 110
KP = LMAX + 2          # contraction rows: 110 x + 2 carry
OFF = np.cumsum([0] + LB)
F32 = mybir.dt.float32
F16 = mybir.dt.float16

# packed stationary tensor [112, 334]: [S0 | Sgen | Slast] (transposed mats)
_WCOLS = 3 * KP - 2


def _dema_mats(alpha: float, beta: float):
    """Per-block stationary matrices M_k [2+L, 112] in float64.

    M_k rows 0:2 produce Z_{k+1}; rows 2:2+L produce the ma rows.
    Contraction cols 0:2 are the carry Z_k, cols 2:112 the x rows (so the
    carry lands at partition base 0 everywhere and the carry placement can
    be a plain engine copy instead of a DMA).
    """
    a, b = float(alpha), float(beta)
    A = np.array([[1.0 - a, 1.0 - a], [-a * b, b * (1.0 - a) + (1.0 - b)]])
    c = np.array([a, a * b])
    e_s = np.array([1.0, 0.0])
    Apow = [np.eye(2)]
    for _ in range(T + 2):
        Apow.append(A @ Apow[-1])
    M0 = np.array([[1.0, 0.0], [-1.0, 1.0]])

    mats = []
    for k, L in enumerate(LB):
        M = np.zeros((2 + L, KP))
        if k == 0:
            zc = Apow[L - 1] @ M0
            M[0:2, 2 + 0] = zc[:, 0]
            M[0:2, 2 + 1] = zc[:, 1]
            for i in range(1, L):
                M[0:2, 2 + i] += Apow[L - 1 - i] @ c
            M[2 + 0, 2 + 0] = 1.0
            for t in range(1, L):
                zc = Apow[t] @ M0
                M[2 + t, 2 + 0] += (e_s @ zc)[0]
                M[2 + t, 2 + 1] += (e_s @ zc)[1]
                for i in range(1, t + 1):
                    M[2 + t, 2 + i] += (Apow[t - i] @ c)[0]
        else:
            if k < NBK - 1:
                for ip in range(L):
                    M[0:2, 2 + ip] = Apow[L - 1 - ip] @ c
                M[0:2, 0:2] = Apow[L]
            for tp in range(L):
                for ip in range(tp + 1):
                    M[2 + tp, 2 + ip] = (Apow[tp - ip] @ c)[0]
                M[2 + tp, 0:2] = e_s @ Apow[tp + 1]
        mats.append(M)
    return mats


def _pack_weights(alpha: float, beta: float) -> np.ndarray:
    mats = _dema_mats(alpha, beta)
    wts = np.zeros((KP, _WCOLS), dtype=np.float16)
    wts[:, 0:KP] = mats[0].T                            # S0   [112, 112]
    wts[:, KP : 2 * KP] = mats[1].T                     # Sgen [112, 112]
    wts[:, 2 * KP : 2 * KP + LMAX] = mats[6].T          # Slast[112, 110]
    return wts


def _wait_limit(inst) -> int:
    # walrus in this container rejects >1 sync wait on several instruction
    # formats (S3_LW, DMA DIRECT2D, CTRL); keep a single wait everywhere
    return 1


class _SplitDrainTC(tile.TileContext):
    """This walrus build rejects more than a couple of sync waits per
    instruction.  After scheduling + the stock kernel-tail drain, walk every
    block and move excess waits onto injected same-engine nops placed
    immediately before the over-limit instruction (waits execute on the
    engine sequencer before dispatch, so this is semantics-preserving)."""

    def _drain_and_barrier(self, tick_clock, wait_clock):
        super()._drain_and_barrier(tick_clock, wait_clock)
        self._split_excess_waits()

    def _split_excess_waits(self):
        nc = self.nc
        cur_list = nc.cur_bb.bb.instructions if nc.cur_bb is not None else None
        for fn in nc.m.functions:
            for blk in fn.blocks:
                insts = blk.instructions
                i = 0
                while i < len(insts):
                    inst = insts[i]
                    si = getattr(inst, "sync_info", None)
                    waits = list(si.on_wait) if si is not None else []
                    limit = _wait_limit(inst)
                    if len(waits) <= limit:
                        i += 1
                        continue
                    keep = waits[:limit]
                    excess = waits[limit:]
                    nops = []
                    for j in range(0, len(excess)):
                        nop = nc.engines[inst.engine].nop(nofuse=True).ins
                        # engine.nop() appended to the current bb; relocate it
                        if cur_list is not None and cur_list and cur_list[-1] is nop:
                            cur_list.pop()
                        nop.sync_info = _bass_rust.SyncInfo(
                            on_wait=excess[j : j + 1], on_update=[]
                        )
                        nops.append(nop)
                    si.on_wait = keep
                    insts[i:i] = nops
                    i += len(nops) + 1


def _dram_ap(dram, k: int, h: int, nrows: int):
    """[nrows (partition=trow), PH*2*C] view of dram[k, 0:nrows, h]."""
    inner = PH * 2 * C
    return bass.AP(
        tensor=dram,
        offset=(k * LMAX * NH + h) * inner,
        ap=[[NH * inner, nrows], [1, inner]],
    )


def _build_nc() -> bass.Bass:
    nc = bass.Bass(trn_type="TRN2", target_bir_lowering=False, debug=False,
                   num_devices=N_CORES)
    # DRAM layout [k, trow, h, q, b', c]: per-(k,h) transfers are
    # [110 partitions x 4KB contiguous] HWDGE-friendly runs; the host does
    # the (cheap) permutation to/from [b, t, c] during shard/unshard.
    # Block 6 trows 108:110 are zero-padded by the host.
    x_d = nc.dram_tensor("x", (NBK, LMAX, NH, PH, 2, C), F16,
                         kind="ExternalInput")
    w_d = nc.dram_tensor("wts", (KP, _WCOLS), F16, kind="ExternalInput")
    z_d = nc.dram_tensor("z0", (2, NPAIR, 2, C), F16, kind="ExternalInput")
    ma_d = nc.dram_tensor("ma", (NBK, LMAX, NH, PH, 2, C), F16,
                          kind="ExternalOutput")

    with _SplitDrainTC(nc) as tc, ExitStack() as ctx:
        const = ctx.enter_context(tc.tile_pool(name="const", bufs=1))
        xbp = ctx.enter_context(tc.tile_pool(name="xb", bufs=NBK * NH))
        masp = ctx.enter_context(tc.tile_pool(name="mas", bufs=10))
        mps = ctx.enter_context(tc.tile_pool(name="mpsum", bufs=8,
                                             space="PSUM"))

        wts = const.tile([KP, _WCOLS], F16)
        s0 = wts[:, 0:KP]                   # block 0: [112, 112]
        sgen = wts[:, KP : 2 * KP]          # blocks 1..5: [112, 112]
        slast = wts[:, 2 * KP : 2 * KP + LMAX]  # block 6: [112, 110]

        # One x tile per block: partitions 0:2 carry Z_k, 2:112 the x rows.
        # All loads ride the SyncE HWDGE ring in order, leading the PE by
        # several blocks.  Block 0's carry rows are zeros (the S0 stationary
        # has zero coefficients there, but SBUF garbage could be NaN and
        # 0*NaN would poison the PE).  Block 0's x is loaded in 4 chunks:
        # region-level hazard tracking lets pair q's first matmul start as
        # soon as its own columns have landed.
        xbs = {}
        for k in range(NBK):
            xbs[k] = xbp.tile([KP, NPAIR, 2, C], F16, tag="xb", name=f"xb{k}")
        nc.scalar.dma_start(xbs[0][0:2, :], z_d.ap())
        nc.sync.dma_start(wts[:], w_d.ap())
        inner = NPAIR * 2 * C
        for lo, hi in ((0, 1), (1, 2), (2, 5), (5, 8)):
            nc.sync.dma_start(
                xbs[0][2:KP, lo:hi],
                bass.AP(tensor=x_d, offset=lo * 2 * C,
                        ap=[[inner, LMAX], [1, (hi - lo) * 2 * C]]),
            )
        for k in range(1, NBK):
            nc.sync.dma_start(
                xbs[k][2:KP, :],
                bass.AP(tensor=x_d, offset=k * LMAX * inner,
                        ap=[[inner, LMAX], [1, inner]]),
            )

        # k-major: 8 matmuls per block (one per pair), PSUM bank = pair.
        # Each pair's PSUM output: rows 0:2 = Z_{k+1}, rows 2:112 = ma.
        # Two copies per pair chase the matmul: the full-tile fp16 copy into
        # mas (round-robin Vector/Scalar/GpSimd) and a tiny carry copy of
        # rows 0:2 straight into the next block's x tile (alternating
        # Vector/Scalar, kept off the engine doing this pair's main copy so
        # it dispatches the moment the matmul retires).  Block k+1's
        # matmuls therefore wait only on 4 tiny copies that complete well
        # inside the other half's matmul window.
        ncop = 0
        ncar = 0
        for k in range(NBK):
            last = k == NBK - 1
            nout = 2 + LB[k]
            xb = xbs[k]
            for h in range(NH):
                mas = masp.tile([KP, PH, 2, C], F16, tag="mas",
                                name=f"mas{k}_{h}")
                for q in range(PH):
                    mp = mps.tile([KP, 2, C], F32, tag="mp")
                    st = s0 if k == 0 else (slast if last else sgen)
                    nc.tensor.matmul(mp[0:nout], st, xb[:, h * PH + q],
                                     start=True, stop=True)
                    # the ONLY PSUM readers are these 28-per-engine main
                    # copies: more PSUM traffic (e.g. per-pair carry reads)
                    # contends with the PE's PSUM writes and slows the
                    # matmuls from ~600ns to ~760ns
                    if ncop % 2 == 0:
                        nc.vector.tensor_copy(mas[0:nout, q], mp[0:nout])
                    else:
                        nc.scalar.copy(mas[0:nout, q], mp[0:nout])
                    ncop += 1
                    if not last and q % 2 == 1:
                        # carry placement at 2-pair granularity, SBUF->SBUF
                        # from mas, on the engine that did NOT run the main
                        # copy that just retired.  The first half's carry
                        # lands while pairs q+1.. still matmul, so the next
                        # block's early matmuls (which only read their own
                        # columns, thanks to region-level hazard tracking)
                        # never wait on the late carry.
                        zdst = xbs[k + 1][0:2, h * PH + q - 1 : h * PH + q + 1]
                        if ncar % 2 == 0:
                            nc.scalar.copy(zdst, mas[0:2, q - 1 : q + 1])
                        else:
                            nc.vector.tensor_copy(zdst, mas[0:2, q - 1 : q + 1])
                        ncar += 1
                # ma store rows 2:112 (DMA is exempt from the 32-aligned
                # partition-base rule); round-robin the three DGE rings (the
                # SyncE ring is drained of loads by the time stores flow)
                dst = _dram_ap(ma_d, k, h, LB[k])
                src = mas[2 : 2 + LB[k], :]
                ring = (k * NH + h) % 3
                if ring == 0:
                    nc.scalar.dma_start(dst, src)
                elif ring == 1:
                    nc.sync.dma_start(dst, src)
                else:
                    nc.gpsimd.dma_start(dst, src)

    return nc


_NC_CACHE: bass.Bass | None = None


def _get_nc() -> bass.Bass:
    global _NC_CACHE
    if _NC_CACHE is None:
        _NC_CACHE = _build_nc()
    return _NC_CACHE


def _tile_layout(x_shard: np.ndarray) -> np.ndarray:
    """[BS, T, C] fp16 -> [NBK, 110, NH, PH, 2, C] tile layout (zero-pad)."""
    v = x_shard.reshape(NH, PH, 2, T, C)
    out = np.zeros((NBK, LMAX, NH, PH, 2, C), dtype=np.float16)
    for k in range(NBK):
        chunk = v[:, :, :, OFF[k] : OFF[k] + LB[k], :]
        out[k, 0 : LB[k]] = chunk.transpose(3, 0, 1, 2, 4)
    return out


def _untile_layout(t: np.ndarray) -> np.ndarray:
    """[NBK, 110, NH, PH, 2, C] -> [BS, T, C] f32."""
    ma = np.empty((NH, PH, 2, T, C), dtype=np.float32)
    for k in range(NBK):
        ma[:, :, :, OFF[k] : OFF[k] + LB[k], :] = (
            t[k, 0 : LB[k]].astype(np.float32).transpose(1, 2, 3, 0, 4)
        )
    return ma.reshape(BS, T, C)


def kernel(x: np.ndarray, alpha, beta):
    x = np.asarray(x, dtype=np.float32)
    assert x.shape == (B, T, C), x.shape
    x16 = x.astype(np.float16)
    wts = _pack_weights(float(alpha), float(beta))

    nc = _get_nc()
    z0 = np.zeros((2, NPAIR, 2, C), dtype=np.float16)
    in_maps = [
        {"x": _tile_layout(x16[i * BS : (i + 1) * BS]), "wts": wts, "z0": z0}
        for i in range(N_CORES)
    ]
    out = run_bass_kernel_spmd(nc, in_maps, core_ids=list(range(N_CORES)))
    ma = np.concatenate(
        [_untile_layout(out.results[i]["ma"]) for i in range(N_CORES)], axis=0
    )
    res = x - ma
    return res, ma
